# revision 29
# baseline (speedup 1.0000x reference)
"""Causal self-attention Trainium2 kernel (8 NeuronCores, bf16 compute).

Sharding: core c -> batch b = c//4, head group hg = c%4 (4 heads each).
Each core computes its heads' QKV projections, causal attention, and a
partial output projection yt[d, t] (transposed). Host sums the 4 partials
per batch, transposes, and adds b_proj.

Device dataflow per core:
  phase X : x [S,D] bf16 -> PE-transpose -> xT tiles [128d, S] resident
  per head: QT/KT/VT = W.T @ xT (transposed projections, hd on partitions)
            V = PE-transpose(VT)  (natural [tok, hd] layout)
            per q-span (512): for each k-block kj:
               ST[k,q] = KT_blk.T @ QT_span   (scores transposed, PSUM)
               += causal mask on diagonal blocks (DVE)
               PT = exp(scale*ST)             (ACT, bf16, unnormalized)
               sum[1,q]  += ones.T @ PT       (PE)
               OT[hd,q]  += V_blk.T @ PT      (PE)
            recipT = 1/sum (DVE), bcast = ones1.T @ recipT (PE rank-1, fp32)
            OT_sbuf = OT * bcast (DVE, bf16)
  proj    : yt[dc, t] += Wp_blk.T @ OT_h  accumulated over heads -> DRAM
"""
import numpy as np

B, S, D, H = 2, 2048, 2048, 16
HD = 128
NCORES = 8
HPC = H // (NCORES // B)     # heads per core = 4
NEG = -1e9


def build_nc(S=S, D=D, nh=HPC, span=512):
    import concourse.bass as bass
    import concourse.mybir as mybir
    from concourse import bacc
    from concourse.tile import TileContext

    f32 = mybir.dt.float32
    bf16 = mybir.dt.bfloat16
    KT = D // 128          # contraction tiles for qkv
    TT = S // 128          # token tiles
    NS = S // span         # q spans
    KPS = span // 128      # k-blocks per span
    scale = float(HD) ** -0.5

    nc = bacc.Bacc("TRN2", target_bir_lowering=False, debug=False)
    x_d = nc.dram_tensor("xt", [D, S], bf16, kind="ExternalInput").ap()
    wq_d = nc.dram_tensor("wqkv", [3 * nh * 128, D], bf16, kind="ExternalInput").ap()
    bq_d = nc.dram_tensor("bqkv", [128, 3 * nh], f32, kind="ExternalInput").ap()
    wp_d = nc.dram_tensor("wproj", [nh * 128, D], bf16, kind="ExternalInput").ap()
    tm_d = nc.dram_tensor("trimaskT", [128, 128], f32, kind="ExternalInput").ap()
    id_d = nc.dram_tensor("identb", [128, 128], bf16, kind="ExternalInput").ap()
    oc_d = nc.dram_tensor("ones_sq", [128, 128], bf16, kind="ExternalInput").ap()
    yt_d = nc.dram_tensor("yt", [D, S], f32, kind="ExternalOutput").ap()

    Act = mybir.ActivationFunctionType
    Alu = mybir.AluOpType

    with TileContext(nc) as tc:
        from contextlib import ExitStack
        with ExitStack() as ctx:
            res = ctx.enter_context(tc.tile_pool(name="res", bufs=1))
            w_p = ctx.enter_context(tc.tile_pool(name="w", bufs=2))
            wp_p = ctx.enter_context(tc.tile_pool(name="wp", bufs=nh))
            qk_p = ctx.enter_context(tc.tile_pool(name="qk", bufs=2))
            v_p = ctx.enter_context(tc.tile_pool(name="v", bufs=2))
            pt_p = ctx.enter_context(tc.tile_pool(name="pt", bufs=4))
            sm_p = ctx.enter_context(tc.tile_pool(name="sm", bufs=4))
            yst_p = ctx.enter_context(tc.tile_pool(name="yst", bufs=2))
            ps_t = ctx.enter_context(tc.tile_pool(name="ps_t", bufs=2, space="PSUM"))
            ps_mm = ctx.enter_context(tc.tile_pool(name="ps_mm", bufs=2, space="PSUM"))
            ps_st = ctx.enter_context(tc.tile_pool(name="ps_st", bufs=4, space="PSUM"))

            # constants
            trimaskT = res.tile([128, 128], f32, tag="trimaskT")
            identb = res.tile([128, 128], bf16, tag="identb")
            ones_sq = res.tile([128, 128], bf16, tag="ones_sq")
            bq = res.tile([128, 3 * nh], f32, tag="bq")
            nc.sync.dma_start(trimaskT, tm_d)
            nc.sync.dma_start(identb, id_d)
            nc.sync.dma_start(ones_sq, oc_d)
            nc.sync.dma_start(bq, bq_d)

            # preload the first two W stripes so qkv h0 isn't gated on them
            prew = {}
            for p in range(2):
                wt0 = w_p.tile([128, D], bf16, tag="w", name=f"w0_{p}")
                nc.sync.dma_start(wt0, wq_d[p * nh * 128:(p * nh + 1) * 128, :])
                prew[(0, p)] = wt0

            # ---- load host-pre-transposed x: xT[kt] [128d, S] stripes,
            # split in half-stripes across DMA rings so early kt land fast ----
            xT = [res.tile([128, S], bf16, tag=f"xt{kt}", name=f"xt{kt}")
                  for kt in range(KT)]
            hS = S // 2
            for kt in range(KT):
                for hh in range(2):
                    nc.sync.dma_start(
                        xT[kt][:, hh * hS:(hh + 1) * hS],
                        x_d[kt * 128:(kt + 1) * 128, hh * hS:(hh + 1) * hS])

            # ---- per-head OT accumulation ----
            OT = [res.tile([128, S], bf16, tag=f"ot{h}", name=f"ot{h}")
                  for h in range(nh)]
            for h in range(nh):
                # qkv projections (transposed: [hd, tok])
                qkvT = []
                for p in range(3):
                    tag = ("qt", "kt_", "vt")[p]
                    dst = qk_p.tile([128, S], bf16, tag=tag)
                    hp = p * nh + h
                    wt = prew.pop((h, p), None)
                    if wt is None:
                        wt = w_p.tile([128, D], bf16, tag="w")
                        nc.sync.dma_start(wt, wq_d[hp * 128:(hp + 1) * 128, :])
                    for sp in range(NS):
                        ps = ps_mm.tile([128, span], f32, tag="mm")
                        for kt in range(KT):
                            nc.tensor.matmul(
                                ps, wt[:, kt * 128:(kt + 1) * 128],
                                xT[kt][:, sp * span:(sp + 1) * span],
                                start=(kt == 0), stop=(kt == KT - 1))
                        nc.scalar.activation(
                            dst[:, sp * span:(sp + 1) * span], ps,
                            Act.Identity, bias=bq[:, hp:hp + 1], scale=1.0)
                    qkvT.append(dst)
                QT, KTt, VT = qkvT

                # V natural [tok, hd]: PE-transpose VT in groups of 4
                vh = v_p.tile([128, S], bf16, tag="v")
                for tg in range(0, TT, 4):
                    n = min(4, TT - tg)
                    ps = ps_t.tile([128, 512], bf16, tag="tp")
                    for j in range(n):
                        nc.tensor.transpose(
                            ps[:, j * 128:(j + 1) * 128],
                            VT[:, (tg + j) * 128:(tg + j + 1) * 128], identb)
                    nc.any.tensor_copy(
                        vh[:, tg * 128:(tg + n) * 128], ps[:, :n * 128])

                # attention per q-span
                for sp in range(NS):
                    nkj = KPS * (sp + 1)   # causal: k-blocks 0..nkj-1
                    ps_o = ps_mm.tile([128, span], f32, tag="mm")
                    ps_s = ps_st.tile([128, span], f32, tag="st")
                    pend = []  # (kj, pt, qoff) awaiting sum/av emission

                    def flush_one():
                        kj, pt, qoff = pend.pop(0)
                        nc.tensor.matmul(
                            ps_s[:, qoff:], ones_sq, pt[:, qoff:],
                            start=(kj == 0), stop=(kj == nkj - 1))
                        nc.tensor.matmul(
                            ps_o[:, qoff:], vh[:, kj * 128:(kj + 1) * 128],
                            pt[:, qoff:], start=(kj == 0), stop=(kj == nkj - 1))

                    for kj in range(nkj):
                        qoff = max(0, (kj - KPS * sp)) * 128
                        ps = ps_st.tile([128, span], f32, tag="st")
                        nc.tensor.matmul(
                            ps[:, qoff:], KTt[:, kj * 128:(kj + 1) * 128],
                            QT[:, sp * span + qoff:(sp + 1) * span],
                            start=True, stop=True)
                        if kj >= KPS * sp:  # diagonal block: causal mask
                            nc.vector.tensor_tensor(
                                out=ps[:, qoff:qoff + 128],
                                in0=ps[:, qoff:qoff + 128],
                                in1=trimaskT, op=Alu.add)
                        pt = pt_p.tile([128, span], bf16, tag="pt")
                        nc.scalar.activation(
                            pt[:, qoff:], ps[:, qoff:], Act.Exp, scale=scale)
                        pend.append((kj, pt, qoff))
                        if len(pend) > 2:
                            flush_one()
                    while pend:
                        flush_one()

                    recipb = sm_p.tile([128, span], f32, tag="recipb")
                    nc.vector.reciprocal_approx_fast(out=recipb, in_=ps_s)
                    nc.vector.tensor_tensor(
                        out=OT[h][:, sp * span:(sp + 1) * span],
                        in0=ps_o, in1=recipb, op=Alu.mult)

            # ---- output projection: yt[dc, t] = sum_h Wp_h.T @ OT_h ----
            wp = []
            for h in range(nh):
                w = wp_p.tile([128, D], bf16, tag="wpt")
                nc.sync.dma_start(w, wp_d[h * 128:(h + 1) * 128, :])
                wp.append(w)
            for dc in range(D // 128):
                yst = yst_p.tile([128, S], f32, tag="yst")
                for sp in range(NS):
                    ps = ps_mm.tile([128, span], f32, tag="mm")
                    for h in range(nh):
                        nc.tensor.matmul(
                            ps, wp[h][:, dc * 128:(dc + 1) * 128],
                            OT[h][:, sp * span:(sp + 1) * span],
                            start=(h == 0), stop=(h == nh - 1))
                    nc.any.tensor_copy(yst[:, sp * span:(sp + 1) * span], ps)
                nc.sync.dma_start(yt_d[dc * 128:(dc + 1) * 128, :], yst)

    nc.finalize()
    return nc


def _prep_core_inputs(x, W_qkv, b_qkv, W_proj, core, S=S, D=D, nh=HPC):
    import ml_dtypes
    bf16 = ml_dtypes.bfloat16
    ngr = NCORES // B
    b, hg = core // ngr, core % ngr
    KT = D // 128
    Dfull = W_qkv.shape[0]

    wq = np.empty((3 * nh * 128, D), dtype=bf16)
    bq = np.zeros((128, 3 * nh), dtype=np.float32)
    for p in range(3):
        for h in range(nh):
            g = hg * nh + h
            col = p * Dfull + g * 128
            blk = W_qkv[:, col:col + 128]            # [D, 128]
            hp = p * nh + h
            wq[hp * 128:(hp + 1) * 128] = (
                blk.reshape(KT, 128, 128).transpose(1, 0, 2).reshape(128, D)
                .astype(bf16))
            bq[:, hp] = b_qkv[col:col + 128]
    wp = W_proj[hg * nh * 128:(hg + 1) * nh * 128, :].astype(bf16)

    r = np.arange(128)
    trimaskT = np.where(r[:, None] <= r[None, :], 0.0, NEG).astype(np.float32)
    return {
        "xt": np.ascontiguousarray(x[b].T).astype(bf16),
        "wqkv": wq,
        "bqkv": bq,
        "wproj": wp,
        "trimaskT": trimaskT,
        "identb": np.eye(128, dtype=bf16),
        "ones_sq": np.ones((128, 128), dtype=bf16),
    }


_CACHE = {}


def kernel(x, W_qkv, b_qkv, W_proj, b_proj, mask):
    from concourse.bass_utils import run_bass_kernel_spmd

    x = np.asarray(x)
    W_qkv = np.asarray(W_qkv)
    b_qkv = np.asarray(b_qkv)
    W_proj = np.asarray(W_proj)
    b_proj = np.asarray(b_proj)

    if "nc" not in _CACHE:
        _CACHE["nc"] = build_nc()
    nc = _CACHE["nc"]

    in_maps = [_prep_core_inputs(x, W_qkv, b_qkv, W_proj, c)
               for c in range(NCORES)]
    res = run_bass_kernel_spmd(nc, in_maps, core_ids=list(range(NCORES)))

    ngr = NCORES // B
    out = np.empty((B, S, D), dtype=np.float32)
    for b in range(B):
        acc = res.results[b * ngr]["yt"].astype(np.float32)
        for g in range(1, ngr):
            acc = acc + res.results[b * ngr + g]["yt"]
        out[b] = acc.T + b_proj[None, :]
    return out
